# revision 34
# baseline (speedup 1.0000x reference)
"""TRN2 Bass kernel for nn_MultiHeadAttention_50835232916148.

Pre-LN multi-head self-attention block (HS=1024, 16 heads, bs=8, sl=1024),
data-parallel over batch across 8 NeuronCores (bs=1 per core, no
collectives).  Per-core dataflow keeps activations in a transposed
[feature, token] layout so every matmul contraction lands on the
partition dim without any on-chip transposes:

  xT --(LN via ones-matmul stats + bcast)--> y [d,t]
  v   = y.T @ WvT    (out [t,e] token layout, +bias, *exp(mask*MASK_NEG))
  per head-pair hp:
    q,k = WqkT.T @ y          (out [e,t]; heads at partition offsets 0/64)
    scoresT_h = kT_h.T @ qT_h (out [tj,ti]; mask folded into v/denominator)
    P = exp(scoresT)          (no max-subtraction; scores are O(1))
    ctx_aug = [v_h | emask].T @ P  (row 64 = softmax denominator)
    ctxn = ctx_aug[0:64] * bcast(1/ctx_aug[64])
  outT = WoutT.T @ ctxn + bias + xT

All matmuls run as float32r (full-rate fp32 PE path, ~1e-4 rel err).
"""

import numpy as np

import concourse.bass as bass
import concourse.mybir as mybir
import concourse.tile as tile
from concourse.bass_utils import run_bass_kernel_spmd

P = 128
HS = 1024
SL = 1024
NHEAD = 16
DH = 64
BS = 8
NT = HS // P          # 8 feature/token tiles
TC = 512              # matmul free-dim chunk (fp32 PSUM bank)
NCH = SL // TC        # 2
LN_EPS = 1e-5
MASK_NEG = -1e8
F32 = mybir.dt.float32
F32R = mybir.dt.float32r
BF16 = mybir.dt.bfloat16
AF = mybir.ActivationFunctionType
ALU = mybir.AluOpType


def _hoist_waits(nc):
    """walrus in this env rejects >1 inline wait per instruction and ANY
    inline wait on Matmult; hoist them onto single-wait NoOps."""
    n_fixed = 0
    for _, bb in nc.bb_map.items():
        inner = bb.bb
        insts = inner.instructions
        new = []
        changed = False
        for inst in insts:
            si = getattr(inst, "sync_info", None)
            if si is not None and si.on_wait:
                keep = 0 if isinstance(inst, mybir.InstMatmult) else 1
                waits = list(si.on_wait)
                if len(waits) > keep:
                    kept = waits[-keep:] if keep else []
                    for w in waits[: len(waits) - keep]:
                        new.append(
                            mybir.InstNoOp(
                                name=nc.get_next_instruction_name(),
                                sync_info=mybir.SyncInfo(on_wait=[w], on_update=[]),
                                bass_nofuse=True,
                                engine=inst.engine,
                            )
                        )
                    inst.sync_info = mybir.SyncInfo(
                        on_wait=kept, on_update=list(si.on_update)
                    )
                    n_fixed += 1
                    changed = True
            new.append(inst)
        if changed:
            inner.instructions = new
    return n_fixed


def _build_nc(hoist=True):
    nc = bass.Bass()

    xt = nc.dram_tensor("xt", [HS, SL], F32R, kind="ExternalInput")
    xtok = nc.dram_tensor("xtok", [SL, HS], F32, kind="ExternalInput")
    epscol = nc.dram_tensor("epscol", [P, 1], F32, kind="ExternalInput")
    wqkv = nc.dram_tensor("wqkv", [HS, 3 * HS], F32R, kind="ExternalInput")
    wout = nc.dram_tensor("wout", [HS, HS], F32R, kind="ExternalInput")
    bqk = nc.dram_tensor("bqk", [P, 16], F32, kind="ExternalInput")
    bvb = nc.dram_tensor("bvb", [P, HS], F32, kind="ExternalInput")
    bo = nc.dram_tensor("bo", [P, NT], F32, kind="ExternalInput")
    emask = nc.dram_tensor("emask", [P, NT], F32, kind="ExternalInput")
    ccol = nc.dram_tensor("ccol", [P, 16], F32R, kind="ExternalInput")
    crow = nc.dram_tensor("crow", [1, 257], F32R, kind="ExternalInput")
    zrows = nc.dram_tensor("zrows", [DH, SL], F32R, kind="ExternalInput")
    out = nc.dram_tensor("out", [HS, SL], F32, kind="ExternalOutput")

    with tile.TileContext(nc) as tc, nc.allow_low_precision(
            reason="float32r tiles feed the fp32r full-rate PE path"):
        with (
            tc.tile_pool(name="big", bufs=1) as big,
            tc.tile_pool(name="wstream", bufs=3) as wstream,
            tc.tile_pool(name="wvs", bufs=8) as wvs,
            tc.tile_pool(name="scratch", bufs=2) as scratch,
            tc.tile_pool(name="qks", bufs=2) as qks,
            tc.tile_pool(name="pts", bufs=3) as pts,
            tc.tile_pool(name="stream", bufs=2) as stream,
            tc.tile_pool(name="vecs", bufs=1) as vecs,
            tc.tile_pool(name="rpool", bufs=1) as rpool,
            tc.tile_pool(name="consts", bufs=1) as consts,
            tc.tile_pool(name="ps_mm", bufs=4, space="PSUM") as ps_mm,
            tc.tile_pool(name="ps_acc", bufs=4, space="PSUM") as ps_acc,
        ):
            # ---- constants & per-core smalls ----
            c_bqk = consts.tile([P, 16], F32, tag="bqk")
            nc.sync.dma_start(c_bqk[:], bqk[:])
            c_bvb = consts.tile([P, HS], F32, tag="bvb")
            c_bo = consts.tile([P, NT], F32, tag="bo")
            nc.sync.dma_start(c_bo[:], bo[:])
            c_em = consts.tile([P, NT], F32, tag="em")
            nc.sync.dma_start(c_em[:], emask[:])
            c_ones = consts.tile([P, 16], F32R, tag="ones")
            nc.sync.dma_start(c_ones[:], ccol[:])
            # +1s/-1s rows, duplicated at partition 0 and partition 64 (the
            # latter feeds the K=1 denominator-broadcast matmul whose rhs
            # lives at partition 64)
            c_row = consts.tile([65, 257], F32R, tag="crow")
            nc.sync.dma_start(c_row[0:1, :], crow[:])
            nc.sync.dma_start(c_row[64:65, :], crow[:])
            c_zero = consts.tile([P, 1], F32, tag="zero")
            nc.vector.memset(c_zero[:], 0.0)

            # ---- big activation tiles ----
            t_y = big.tile([P, NT, SL], F32R, tag="y")

            # ================= Phase 1: LayerNorm =================
            # Stats from a token-major copy of x: free-dim DVE reductions
            # (128 lanes) instead of M=1 PE matmuls + single-lane vector ops.
            c_eps = consts.tile([P, 1], F32, tag="eps")
            nc.sync.dma_start(c_eps[:], epscol[:])
            sm = vecs.tile([P, NT], F32, tag="sm")
            ss = vecs.tile([P, NT], F32, tag="ss")
            for i in range(NT):
                xk = stream.tile([P, HS], F32, tag="t1", bufs=3, name=f"xk{i}")
                nc.sync.dma_start(xk[:], xtok[i * P:(i + 1) * P, :])
                sqs = scratch.tile([P, HS], F32, tag="scr", name=f"sqs{i}")
                nc.scalar.activation(sqs[:], xk[:], AF.Square,
                                     accum_out=ss[:, i:i + 1])
                nc.scalar.activation(sqs[:], xk[:], AF.Copy,
                                     accum_out=sm[:, i:i + 1])
            t_m2 = vecs.tile([P, NT], F32, tag="m2")
            nc.scalar.activation(t_m2[:], sm[:], AF.Square, scale=1.0 / HS)
            t_msq = vecs.tile([P, NT], F32, tag="tmsq")
            nc.scalar.activation(t_msq[:], ss[:], AF.Copy, scale=1.0 / HS)
            t_mean = vecs.tile([P, NT], F32R, tag="tmean")
            nc.scalar.activation(t_mean[:], sm[:], AF.Copy, scale=1.0 / HS)
            nc.vector.tensor_sub(t_msq[:], t_msq[:], t_m2[:])   # -> var
            nc.scalar.activation(t_msq[:], t_msq[:], AF.Ln, bias=c_eps[:])
            t_istd = vecs.tile([P, NT], F32R, tag="tistd")
            nc.scalar.activation(t_istd[:], t_msq[:], AF.Exp, scale=-0.5,
                                 bias=c_zero[:])
            nc.vector.tensor_mul(t_mean[:], t_mean[:], t_istd[:])  # -> m*istd
            # shift per-partition stats into free-dim row vectors (DMA can
            # cross partitions; engines cannot): t = i*128 + p
            v_istd = vecs.tile([1, SL], F32R, tag="mean")
            v_b2 = vecs.tile([1, SL], F32R, tag="msq")
            for i in range(NT):
                nc.sync.dma_start(v_istd[0:1, i * P:(i + 1) * P],
                                  t_istd[:, i:i + 1])
                nc.sync.dma_start(v_b2[0:1, i * P:(i + 1) * P],
                                  t_mean[:, i:i + 1])

            t_A = scratch.tile([P, SL], F32, tag="scr")   # bcast(invstd)
            t_B = scratch.tile([P, SL], F32, tag="scr")   # bcast(-mean*invstd)
            for n in range(NCH):
                sl_ = slice(n * TC, (n + 1) * TC)
                pA = ps_mm.tile([P, TC], F32, tag="mm")
                nc.tensor.matmul(pA[:], c_row[0:1, 0:P], v_istd[:, sl_],
                                 start=True, stop=True)
                nc.scalar.activation(t_A[:, sl_], pA[:], AF.Copy)
                pB = ps_mm.tile([P, TC], F32, tag="mm")
                nc.tensor.matmul(pB[:], c_row[0:1, P:2 * P], v_b2[:, sl_],
                                 start=True, stop=True)
                nc.scalar.activation(t_B[:, sl_], pB[:], AF.Copy)

            for i in range(NT):
                xti = stream.tile([P, SL], F32R, tag="t1", bufs=3, name=f"xtb{i}")
                nc.sync.dma_start(xti[:], xt[i * P:(i + 1) * P, :])
                t1 = stream.tile([P, SL], F32, tag="t1", bufs=3)
                nc.vector.tensor_mul(t1[:], xti[:], t_A[:])
                nc.vector.tensor_add(t_y[:, i, :], t1[:], t_B[:])

            # ============ Phase 2: V projection (token layout) ==========
            # t_vaug reuses t_xt's slot (xt re-read later via small DMAs)
            nc.sync.dma_start(c_bvb[:], bvb[:])
            t_vaug = big.tile([P, NT, NHEAD * (DH + 1)], BF16, tag="vaug")
            for n in range(NCH):
                wv_tiles = []
                for k in range(NT):
                    wv = wvs.tile([P, TC], F32R, tag="wv")
                    nc.sync.dma_start(
                        wv[:], wqkv[k * P:(k + 1) * P,
                                    2 * HS + n * TC:2 * HS + (n + 1) * TC])
                    wv_tiles.append(wv)
                for i in range(NT):
                    ps = ps_mm.tile([P, TC], F32, tag="mm")
                    for k in range(NT):
                        nc.tensor.matmul(
                            ps[:], t_y[:, k, i * P:(i + 1) * P], wv_tiles[k][:],
                            start=(k == 0), stop=(k == NT - 1))
                    vd = stream.tile([P, TC], F32, tag="vd")
                    nc.vector.tensor_add(vd[:], ps[:], c_bvb[:, n * TC:(n + 1) * TC])
                    dst = t_vaug[:, i, 8 * n * (DH + 1):(8 * n + 8) * (DH + 1)]
                    dst = dst.rearrange("p (h c) -> p h c", c=DH + 1)[:, :, 0:DH]
                    nc.vector.tensor_scalar_mul(
                        dst, vd[:].rearrange("p (h c) -> p h c", c=DH),
                        c_em[:, i:i + 1])
            for i in range(NT):
                dst = t_vaug[:, i, :].rearrange("p (h c) -> p h c", c=DH + 1)
                nc.vector.tensor_scalar_mul(
                    dst[:, :, DH:DH + 1],
                    c_ones[:, 0:16].rearrange("p (h c) -> p h c", c=1),
                    c_em[:, i:i + 1])

            # ========= Phase 3+4: per head-pair QK proj + attention =====
            t_ctxn = big.tile([P, NT, SL], F32R, tag="ctxn")

            def normalize(hp, ctx_ps):
                # ctxn = ctx[0:64] * bcast(1/ctx[64]).
                # The denominator sits at partition 64, so the reciprocal is
                # computed in-lane at partition 64 and broadcast down to
                # partitions 0..63 by a K=1 matmul.  Odd heads' result must
                # land at partitions 64..127 of ctxn; engines cannot shift
                # partitions, so those go through a small SBUF->SBUF DMA.
                rs = []
                for hh in range(2):
                    for n in range(NCH):
                        r = rpool.tile([DH + 1, TC], F32R, tag="r", bufs=4,
                                       name=f"r{hp}_{hh}_{n}")
                        nc.scalar.activation(r[DH:DH + 1, :],
                                             ctx_ps[hh][n][DH:DH + 1, :],
                                             AF.Ln, bias=c_zero[DH:DH + 1, :])
                        rs.append(r)
                for hh in range(2):
                    for n in range(NCH):
                        r = rs[hh * NCH + n]
                        nc.scalar.activation(r[DH:DH + 1, :], r[DH:DH + 1, :],
                                             AF.Exp, scale=-1.0,
                                             bias=c_zero[DH:DH + 1, :])
                rbs = []
                for hh in range(2):
                    for n in range(NCH):
                        ps_rb = ps_mm.tile([DH, TC], F32, tag="mm",
                                           name=f"psrb{hp}_{hh}_{n}")
                        nc.tensor.matmul(ps_rb[:], c_row[64:65, 0:DH],
                                         rs[hh * NCH + n][DH:DH + 1, :],
                                         start=True, stop=True)
                        rbs.append(ps_rb)
                for hh in range(2):
                    for n in range(NCH):
                        sl_ = slice(n * TC, (n + 1) * TC)
                        ps_rb = rbs[hh * NCH + n]
                        rb = stream.tile([DH, TC], F32, tag="rb",
                                         name=f"rb{hp}_{hh}_{n}")
                        nc.vector.tensor_copy(rb[:], ps_rb[:])
                        if hh == 0:
                            nc.vector.tensor_mul(
                                t_ctxn[0:DH, hp, sl_],
                                ctx_ps[hh][n][0:DH, :], rb[:])
                        else:
                            cs = stream.tile([DH, TC], F32R, tag="cs",
                                             name=f"cs{hp}_{n}")
                            nc.vector.tensor_mul(cs[:], ctx_ps[hh][n][0:DH, :],
                                                 rb[:])
                            nc.sync.dma_start(t_ctxn[DH:P, hp, sl_], cs[:])

            pending = None
            for hp in range(NHEAD // 2):
                qb = qks.tile([P, SL], F32R, tag="qb")
                # Per-head k tiles, zero-padded on the other head's partitions
                # so scores run as full K=128 matmuls (keeps the PE weight
                # double-buffer engaged; K=64 row-tiled matmuls serialize the
                # fp32r weight load and cost ~2x).
                kz = [qks.tile([P, SL], F32R, tag="kb", name=f"kz{hp}_{hh}")
                      for hh in range(2)]
                nc.sync.dma_start(kz[0][DH:P, :], zrows[:])
                nc.sync.dma_start(kz[1][0:DH, :], zrows[:])
                for blk, is_k in ((hp, False), (8 + hp, True)):
                    wj = wstream.tile([P, NT, P], F32R, tag="wqk",
                                      name=f"wj{hp}_{int(is_k)}")
                    nc.sync.dma_start(
                        wj[:], wqkv[:, blk * P:(blk + 1) * P]
                        .rearrange("(n p) m -> p n m", p=P))
                    if is_k and pending is not None:
                        normalize(*pending)
                        pending = None
                    for n in range(NCH):
                        sl_ = slice(n * TC, (n + 1) * TC)
                        ps = ps_mm.tile([P, TC], F32, tag="mm",
                                        name=f"qk{hp}_{int(is_k)}_{n}")
                        for i in range(NT):
                            nc.tensor.matmul(ps[:], wj[:, i, :], t_y[:, i, sl_],
                                             start=(i == 0), stop=(i == NT - 1))
                        if is_k:
                            nc.vector.tensor_scalar_add(
                                kz[0][0:DH, sl_], ps[0:DH, :],
                                c_bqk[0:DH, blk:blk + 1])
                            nc.vector.tensor_scalar_add(
                                kz[1][DH:P, sl_], ps[DH:P, :],
                                c_bqk[DH:P, blk:blk + 1])
                        else:
                            nc.vector.tensor_scalar_add(
                                qb[:, sl_], ps[:], c_bqk[:, blk:blk + 1])

                ctx_ps = [[ps_acc.tile([DH + 1, TC], F32, tag="acc",
                                        name=f"ctx{hp}_{hh}_{n}")
                           for n in range(NCH)] for hh in range(2)]
                for jt in range(NT):
                    for hh in range(2):
                        h = 2 * hp + hh
                        pt = pts.tile([P, SL], BF16, tag="p")
                        for n in range(NCH):
                            sl_ = slice(n * TC, (n + 1) * TC)
                            ps_s = ps_mm.tile([P, TC], F32, tag="mm",
                                              name=f"s{hp}_{jt}_{hh}_{n}")
                            nc.tensor.matmul(
                                ps_s[:],
                                kz[hh][:, jt * P:(jt + 1) * P],
                                qb[:, sl_],
                                start=True, stop=True)
                            nc.scalar.activation(pt[:, sl_], ps_s[:], AF.Exp,
                                                 bias=c_zero[:])
                        va = t_vaug[:, jt, h * (DH + 1):(h + 1) * (DH + 1)]
                        for n in range(NCH):
                            sl_ = slice(n * TC, (n + 1) * TC)
                            nc.tensor.matmul(ctx_ps[hh][n][:], va, pt[:, sl_],
                                             start=(jt == 0), stop=(jt == NT - 1))
                pending = (hp, ctx_ps)
            normalize(*pending)

            # ================= Phase 5: out-proj + residual =============
            for j in range(NT):
                wo = wstream.tile([P, NT, P], F32R, tag="wqk")
                nc.sync.dma_start(
                    wo[:], wout[:, j * P:(j + 1) * P]
                    .rearrange("(n p) m -> p n m", p=P))
                for n in range(NCH):
                    sl_ = slice(n * TC, (n + 1) * TC)
                    ps = ps_mm.tile([P, TC], F32, tag="mm")
                    for k in range(NT):
                        nc.tensor.matmul(ps[:], wo[:, k, :], t_ctxn[:, k, sl_],
                                         start=(k == 0), stop=(k == NT - 1))
                    od = stream.tile([P, TC], F32, tag="od")
                    nc.scalar.activation(od[:], ps[:], AF.Identity,
                                         bias=c_bo[:, j:j + 1])
                    xr = stream.tile([P, TC], F32R, tag="vd")
                    nc.sync.dma_start(xr[:], xt[j * P:(j + 1) * P, sl_])
                    ot = stream.tile([P, TC], F32, tag="ot")
                    nc.vector.tensor_add(ot[:], od[:], xr[:])
                    nc.sync.dma_start(out[j * P:(j + 1) * P, sl_], ot[:])

    if hoist:
        _hoist_waits(nc)
    return nc


_NC_CACHE = None


def _get_nc():
    global _NC_CACHE
    if _NC_CACHE is None:
        _NC_CACHE = _build_nc()
    return _NC_CACHE


def _prep_in_maps(hidden_states, encoder_padding_mask, in_proj_weight,
                  in_proj_bias, out_proj_weight, out_proj_bias,
                  norm_weight, norm_bias):
    f = np.float32
    w2 = np.asarray(in_proj_weight, dtype=f).reshape(3 * HS, HS).copy()
    b2 = np.asarray(in_proj_bias, dtype=f).reshape(3 * HS).copy()
    # fold the LN affine (w, b) into the fused projection: W*(y*w+b)+bias
    # == (W*diag(w))*y + (bias + W@b)
    nw = np.asarray(norm_weight, dtype=f).reshape(HS)
    nb = np.asarray(norm_bias, dtype=f).reshape(HS)
    b2 = b2 + w2 @ nb
    w2 = w2 * nw[None, :]
    scale = f(1.0 / np.sqrt(DH))
    w2[0:HS] *= scale
    b2[0:HS] *= scale
    wqkv = np.ascontiguousarray(w2.T)                      # [d, 3HS]
    wout = np.ascontiguousarray(np.asarray(out_proj_weight, dtype=f).T)
    bqk = np.ascontiguousarray(b2[:2 * HS].reshape(16, P).T)
    bvb = np.ascontiguousarray(np.broadcast_to(b2[2 * HS:], (P, HS)))
    bo = np.ascontiguousarray(np.asarray(out_proj_bias, dtype=f).reshape(NT, P).T)
    ccol = np.ones((P, 16), dtype=f)
    crow = np.concatenate([np.ones((1, P), f), -np.ones((1, P), f),
                           np.full((1, 1), LN_EPS, f)], axis=1)
    zrows = np.zeros((DH, SL), dtype=f)
    shared = dict(wqkv=wqkv, wout=wout, bqk=bqk, bvb=bvb, bo=bo,
                  ccol=ccol, crow=crow, zrows=zrows)

    hs = np.asarray(hidden_states, dtype=f)
    mask = np.asarray(encoder_padding_mask)
    in_maps = []
    for c in range(BS):
        em = np.exp(mask[c].astype(f) * f(MASK_NEG)).astype(f)
        in_maps.append(dict(
            xt=np.ascontiguousarray(hs[c].T),
            xtok=np.ascontiguousarray(hs[c]),
            epscol=np.full((P, 1), LN_EPS, dtype=f),
            emask=np.ascontiguousarray(em.reshape(NT, P).T),
            **shared,
        ))
    return in_maps


def _run(in_maps, trace=False):
    nc = _get_nc()
    return run_bass_kernel_spmd(nc, in_maps, list(range(BS)), trace=trace)


def kernel(**inputs):
    in_maps = _prep_in_maps(**inputs)
    res = _run(in_maps, trace=False)
    outs = [res.results[c]["out"].T for c in range(BS)]
    return np.stack(outs, axis=0).astype(np.float32)


def kernel_traced(**inputs):
    in_maps = _prep_in_maps(**inputs)
    res = _run(in_maps, trace=True)
    outs = [res.results[c]["out"].T for c in range(BS)]
    return np.stack(outs, axis=0).astype(np.float32), res.exec_time_ns
